# revision 2
# baseline (speedup 1.0000x reference)
"""CARAFE kernel for 8x Trainium2 NeuronCores — PE-based reassembly, v2.

Core = b*4 + q (batch, H-quarter). Per core:
  comp 1x1 conv + enc 3x3 conv (bf16) produce per-(row-pair lam) tap
  weights E; exp with accum_out gives softmax sums.
  E is DMA-scattered into a lam-major DRAM scratch
  [lam][ki-plane][jt-slot][rho][j][dd] so that
    - one scatter DMA per (lam-pair, kj) covers both lams (the (lam,ki)
      dims fuse into one stride-S_A dim because LAM == 5*S_A), and
    - one gather DMA per (half, rt, rho, role-slot) loads the banded
      reassembly matrices for 4 lams at once with 512B-contiguous runs.
  Zero cells come from the host-zeroed scratch; scratch is written once
  (no rotation), so guards stay valid.
  Reassembly: per (lam, dd) out tile, 3 accumulating PE matmuls
  out[(rho,j), c] += B_slot^T @ x_rowpair, one per source row-pair slot.
  Softmax normalization happens in the PSUM->SBUF eviction on DVE
  (multiply by the per-partition reciprocal, broadcast over channels).
  Out DMAs ride the Pool/SWDGE path to keep HWDGE free.
"""

import sys

if "/opt/trn_rl_repo" not in sys.path:
    sys.path.insert(0, "/opt/trn_rl_repo")

import numpy as np
import ml_dtypes

BF16 = ml_dtypes.bfloat16

SCALE = 2
K_UP = 5
EPS = 1e-5
B, C, H, W = 2, 256, 64, 64
CM = 64
NK = 100
NCORES = 8
QH = 16
NW1R = 18

# scratch geometry (elements):
# cell addr = lam_local*LAM + ki*S_A + (j+kj)*S_JT + rho*S_PR + j*S_J + dd
# S_PR = 64*(S_JT+S_J) makes the scatter's 128 (rho,j) partitions one
# uniform-stride-516 dim; LAM = 5*S_A makes (lam,ki) one stride-S_A dim.
S_J = 4
S_JT = 512
S_PR = 64 * (S_JT + S_J)     # 33024
S_A = 68096                  # > S_PR + 67*S_JT + 63*S_J + 3 = 67583
LAM = 5 * S_A                # 340480
# unequal halves: a small first half starts its reassembly while the
# big half's scatters are still streaming through the DMA engines
HALVES = [(0, 4), (4, 8)]    # (first lam, end lam)

_compiled = {}


def _build_nc():
    import concourse.bacc as bacc
    import concourse.bass as bass
    import concourse.mybir as mybir
    import concourse.tile as tile
    from concourse.tile_rust import add_dep_helper

    f32 = mybir.dt.float32
    bf16 = mybir.dt.bfloat16
    nc = bacc.Bacc("TRN2", target_bir_lowering=False, debug=False)

    x_nat_d = nc.dram_tensor("x_nat", [128, 2, NW1R, W], bf16, kind="ExternalInput")
    x_pix_d = nc.dram_tensor("x_pix", [128, 10, C], bf16, kind="ExternalInput")
    # pack: [0:128) comp_lhsT (2,64) | [128:428) enc_pair (3,100) |
    # [428:728) enc_single rows 0-63 | [728:828) enc_bias row 0 |
    # [828:846) rmask rows 0-63
    pack_d = nc.dram_tensor("pack", [128, 846], bf16, kind="ExternalInput")
    comp_bias_d = nc.dram_tensor("comp_bias", [CM, 1], f32, kind="ExternalInput")
    scr_ds = [
        nc.dram_tensor(f"scr{i}", [1, (l1 - l0) * LAM], bf16, kind="ExternalOutput")
        for i, (l0, l1) in enumerate(HALVES)
    ]
    out_d = nc.dram_tensor("out", [8, 128, 4, C], bf16, kind="ExternalOutput")

    def scr_ap(which, off_els, dims):
        sl = scr_ds[which][0:1, off_els : off_els + 1]
        return bass.AP(
            tensor=sl.tensor,
            offset=sl.offset,
            ap=[[1, 1]] + [list(d) for d in dims],
        )

    with tile.TileContext(nc) as tc:
        with (
            tc.tile_pool(name="consts", bufs=1) as consts,
            tc.tile_pool(name="big", bufs=1) as big,
            tc.tile_pool(name="epool", bufs=1) as epool,
            tc.tile_pool(name="bpool", bufs=1) as bpool,
            tc.tile_pool(name="opool", bufs=4) as opool,
        ):
            # ---- input loads on Pool/SWDGE (keeps the HWDGE rings free
            # for the scatter/gather round trip); x_nat first (split by
            # h so comp conv's first matmul starts earlier): comp conv
            # is the head of the critical path ----
            x_nat = big.tile([128, 2, NW1R, W], bf16, tag="x_nat")
            nc.gpsimd.dma_start(x_nat, x_nat_d[:])
            comp_bias = consts.tile([CM, 1], f32, tag="comp_bias")
            nc.gpsimd.dma_start(comp_bias, comp_bias_d[:])
            pack = consts.tile([128, 846], bf16, tag="pack")
            nc.sync.dma_start(pack, pack_d[:])
            x_pix = big.tile([128, 10, C], bf16, tag="x_pix")
            nc.gpsimd.dma_start(x_pix, x_pix_d[:])
            comp_lhsT = pack[:, 0:128].rearrange("p (h c) -> p h c", c=CM)
            enc_pair = pack[:, 128:428].rearrange("p (t k) -> p t k", k=NK)
            enc_single = pack[0:CM, 428:728].rearrange("p (t k) -> p t k", k=NK)
            enc_bias = pack[0:1, 728:828]
            rmask = pack[0:CM, 828:846]
            ones_row = consts.tile([1, 128], bf16, tag="ones_row")
            nc.vector.memset(ones_row, 1.0)

            # ---- B tiles, one per half: [128(rt,jt), lam nl, slot 3,
            # rho 2, j 64, dd 4]. Gathers overwrite every cell except the
            # two never-loaded (rt,rho,slot) quadrants -> memset those. ----
            btiles = []
            for h, (l0, l1) in enumerate(HALVES):
                bth = bpool.tile([128, l1 - l0, 3, 2, W, 4], bf16, tag=f"B_{h}")
                btiles.append(bth)
            for bt in btiles:
                nc.vector.memset(bt[0:64, :, 0, 1], 0.0)
                nc.vector.memset(bt[64:128, :, 2, 0], 0.0)

            S = big.tile([128, 8, 4], f32, tag="S")
            S_f = S.rearrange("p l q -> p (l q)")
            R = big.tile([128, 8, 4], f32, tag="R")
            R_f = R.rearrange("p l q -> p (l q)")

            # ---- comp 1x1 conv -> W1, in 3 row-blocks of 6 so the enc
            # convs (and therefore scatters) start before the whole comp
            # conv is done. rmask (valid-row mask) is folded into the
            # shifted-copy ops, split across DVE and Pool. ----
            def rmask_bc(r0, r1, n):
                sl = rmask[:, r0:r1]
                return bass.AP(
                    tensor=sl.tensor,
                    offset=sl.offset,
                    ap=[list(sl.ap[0]), list(sl.ap[1]), [0, n]],
                )

            w1c = big.tile([CM, NW1R, W], bf16, tag="w1c")
            w1main = big.tile([128, NW1R, W], bf16, tag="w1main")
            nc.vector.memset(w1main[0:CM, :, 0:1], 0.0)
            w1sing = big.tile([CM, NW1R, W], bf16, tag="w1sing")
            nc.vector.memset(w1sing[:, :, W - 1 : W], 0.0)

            x_nat_f = x_nat.rearrange("p h r w -> p h (r w)")
            with tc.tile_pool(name="pcomp", bufs=1, space="PSUM") as pcomp:
                for blk in range(3):
                    r0, r1 = 6 * blk, 6 * blk + 6
                    n0 = r0 * W
                    psum_cb = pcomp.tile([CM, 6 * W], f32, tag=f"psc_{blk}")
                    for h in range(2):
                        nc.tensor.matmul(
                            psum_cb,
                            lhsT=comp_lhsT[:, h, :],
                            rhs=x_nat_f[:, h, n0 : n0 + 6 * W],
                            start=(h == 0),
                            stop=(h == 1),
                        )
                    psum_cb_v = psum_cb.rearrange("p (r w) -> p r w", w=W)
                    nc.scalar.activation(
                        out=w1c[:, r0:r1, :],
                        in_=psum_cb_v,
                        func=mybir.ActivationFunctionType.Relu,
                        bias=comp_bias,
                        scale=1.0,
                    )
                    nc.gpsimd.tensor_tensor(
                        out=w1main[CM:128, r0:r1],
                        in0=w1c[:, r0:r1],
                        in1=rmask_bc(r0, r1, W),
                        op=mybir.AluOpType.mult,
                    )
                    nc.vector.tensor_tensor(
                        out=w1main[0:CM, r0:r1, 1:W],
                        in0=w1c[:, r0:r1, 0 : W - 1],
                        in1=rmask_bc(r0, r1, W - 1),
                        op=mybir.AluOpType.mult,
                    )
                    nc.gpsimd.tensor_tensor(
                        out=w1sing[:, r0:r1, 0 : W - 1],
                        in0=w1c[:, r0:r1, 1:W],
                        in1=rmask_bc(r0, r1, W - 1),
                        op=mybir.AluOpType.mult,
                    )
            w1main_f = w1main.rearrange("p r w -> p (r w)")
            w1sing_f = w1sing.rearrange("p r w -> p (r w)")

            # ---- enc conv + exp per row pair lam; E for a lam QUAD in
            # one tile so a single scatter DMA spans 4 lams ----
            E_gs = []
            for g, (l0, l1) in enumerate(HALVES):
                Egt = epool.tile([128, l1 - l0, NK], bf16, tag=f"E_{g}")
                E_gs.append(Egt)
            with tc.tile_pool(name="penc", bufs=3, space="PSUM") as penc:
                for lam in range(8):
                    psum_e = penc.tile([128, NK], f32, tag="psum_e")
                    for ty in range(3):
                        o = (2 * lam + ty) * W
                        nc.tensor.matmul(
                            psum_e,
                            lhsT=w1main_f[:, o : o + 2 * W],
                            rhs=enc_pair[:, ty, :],
                            start=(ty == 0),
                            stop=False,
                        )
                    for ty in range(3):
                        o = (2 * lam + ty) * W
                        nc.tensor.matmul(
                            psum_e,
                            lhsT=w1sing_f[:, o : o + 2 * W],
                            rhs=enc_single[:, ty, :],
                            start=False,
                            stop=False,
                        )
                    nc.tensor.matmul(
                        psum_e, lhsT=ones_row, rhs=enc_bias, start=False, stop=True
                    )

                    # exp, then per-dd softmax sums on DVE (f32 accum)
                    hh = next(i for i, (a0, a1) in enumerate(HALVES) if lam < a1)
                    Eg = E_gs[hh]
                    E = Eg[:, lam - HALVES[hh][0], :]
                    nc.scalar.activation(
                        out=E,
                        in_=psum_e,
                        func=mybir.ActivationFunctionType.Exp,
                    )
                    E_v = E.rearrange("p (t d) -> p d t", d=4)
                    nc.vector.tensor_reduce(
                        out=S[:, lam, :],
                        in_=E_v,
                        axis=mybir.AxisListType.X,
                        op=mybir.AluOpType.add,
                    )
                    nc.vector.reciprocal(
                        out=R_f[:, lam * 4 : lam * 4 + 4],
                        in_=S_f[:, lam * 4 : lam * 4 + 4],
                    )

            # ---- scatter/gather round trip. One scatter DMA per
            # (half, kj) spans 4 lams via the fused (lam,ki) dim; one
            # gather DMA per (half, slot, rt, rho) loads 4 lams at once,
            # slot-major so the reassembly can start after the first
            # 3-4 DMAs. All scatters + h1 gathers ride the SP queue
            # (same-queue HWDGE FIFO orders h1's RAW hazard); h0
            # gathers ride Act with explicit scatter->gather dep edges. ----
            def emit_scatters(h2, eng):
                insts = []
                Eg = E_gs[h2]
                nl = HALVES[h2][1] - HALVES[h2][0]
                for kj in range(K_UP):
                    src = bass.AP(
                        tensor=Eg.tensor,
                        offset=Eg.offset + kj * 4,
                        ap=[list(Eg.ap[0]), [20, 5 * nl], [1, 4]],
                    )
                    dst = scr_ap(
                        h2,
                        kj * S_JT,
                        [[S_JT + S_J, 128], [S_A, 5 * nl], [1, 4]],
                    )
                    insts.append(eng.dma_start(dst, src))
                return insts

            def emit_gathers(h2, dep_insts, order_insts):
                # alternate SP/Act per DMA so each half's 10 gather
                # issues split across both HWDGE queues
                bt = btiles[h2]
                nl = HALVES[h2][1] - HALVES[h2][0]
                insts = []
                k = 0
                for s in range(3):
                    for rt in range(2):
                        for rho in range(2):
                            a = 2 * s + rt - rho  # ki plane
                            if a < 0 or a >= K_UP:
                                continue
                            # dim order must match the dst iteration:
                            # (jt partitions, lam, (j,dd) run)
                            src = scr_ap(
                                h2,
                                a * S_A + 2 * S_JT + rho * S_PR,
                                [[S_JT, W], [LAM, nl], [1, W * 4]],
                            )
                            eng = nc.sync if k % 2 == 0 else nc.scalar
                            k += 1
                            gi = eng.dma_start(
                                bt[64 * rt : 64 * rt + 64, :, s, rho], src
                            )
                            insts.append(gi)
                            for si in dep_insts:
                                add_dep_helper(
                                    gi.ins, si.ins, sync=True, reason="scr RAW"
                                )
                            for si in order_insts:
                                add_dep_helper(
                                    gi.ins, si.ins, sync=False,
                                    reason="ring order",
                                )
                return insts

            # global device order sc0 < sc1 < g0 < g1 (matches completion
            # order and keeps the HWDGE ring slots clean)
            sc0 = emit_scatters(0, nc.sync)
            sc1 = emit_scatters(1, nc.sync)
            emit_gathers(0, sc0, sc1)
            emit_gathers(1, sc1, [])

            # ---- reassembly (PE) + eviction (DVE) + out (Pool/SWDGE).
            # Slot-major matmul order matches gather arrival; psum region
            # per (lam, dd-pair) accumulates across the 3 slots. ----
            with tc.tile_pool(name="pout", bufs=1, space="PSUM") as pout:
                for h2 in range(2):
                    bt = btiles[h2]
                    l0, l1 = HALVES[h2]
                    nl = l1 - l0
                    groups = [
                        tuple(range(g, min(g + 2, nl))) for g in range(0, nl, 2)
                    ]
                    pos = {}
                    ots = {}
                    for grp in groups:
                      for s in range(3):
                        for li in grp:
                            lam = l0 + li
                            for dd in range(4):
                                if s == 0:
                                    po_t = pout.tile(
                                        [128, C], f32, tag=f"po_{li % 2}_{dd}"
                                    )
                                    pos[(li, dd)] = po_t
                                po = pos[(li, dd)]
                                lhsT = bt[:, li, s, :, :, dd].rearrange(
                                    "p r j -> p (r j)"
                                )
                                nc.tensor.matmul(
                                    po,
                                    lhsT=lhsT,
                                    rhs=x_pix[:, lam + s, :],
                                    start=(s == 0),
                                    stop=(s == 2),
                                )
                            if s == 2:
                                # all 4 dd of this lam finished: evict with
                                # per-partition softmax normalization
                                # evictions split across Act and DVE so the
                                # psum tags recycle at 2x rate; out DMA per
                                # dd-pair to shorten the last-lam tail
                                ot = opool.tile([128, 4, C], bf16, tag="ot")
                                ots[li] = ot
                                for dd in range(4):
                                    idx = lam * 4 + dd
                                    if dd % 2 == 0:
                                        nc.scalar.activation(
                                            out=ot[:, dd, :],
                                            in_=pos[(li, dd)],
                                            func=mybir.ActivationFunctionType.Copy,
                                            scale=R_f[:, idx : idx + 1],
                                        )
                                    else:
                                        r_sl = R_f[:, idx : idx + 1]
                                        r_b = bass.AP(
                                            tensor=r_sl.tensor,
                                            offset=r_sl.offset,
                                            ap=[list(r_sl.ap[0]), [0, C]],
                                        )
                                        nc.vector.tensor_tensor(
                                            out=ot[:, dd, :],
                                            in0=pos[(li, dd)],
                                            in1=r_b,
                                            op=mybir.AluOpType.mult,
                                        )
                                    if dd == 3:
                                        oeng = (
                                            nc.sync if lam % 2 == 0 else nc.scalar
                                        )
                                        oeng.dma_start(out_d[lam], ot)

    nc.compile()
    return nc


def _host_inputs(x, comp_w, comp_gamma, comp_beta, comp_mean, comp_var,
                 enc_w, enc_gamma, enc_beta, enc_mean, enc_var):
    x = np.asarray(x, dtype=np.float32)
    scale_c = (np.asarray(comp_gamma) / np.sqrt(np.asarray(comp_var) + EPS)).astype(
        np.float32
    )
    bias_c = (np.asarray(comp_beta) - np.asarray(comp_mean) * scale_c).astype(
        np.float32
    )
    wp = np.asarray(comp_w)[:, :, 0, 0].astype(np.float32) * scale_c[:, None]
    comp_lhsT = np.ascontiguousarray(
        wp.T.reshape(2, 128, CM).transpose(1, 0, 2)
    ).astype(BF16)
    comp_bias = bias_c.reshape(CM, 1)

    scale_e = (np.asarray(enc_gamma) / np.sqrt(np.asarray(enc_var) + EPS)).astype(
        np.float32
    )
    bias_e = (np.asarray(enc_beta) - np.asarray(enc_mean) * scale_e).astype(np.float32)
    ew = np.asarray(enc_w).astype(np.float32) * scale_e[:, None, None, None]
    enc_pair = np.zeros((128, 3, NK), np.float32)
    enc_single = np.zeros((CM, 3, NK), np.float32)
    for ty in range(3):
        enc_pair[0:CM, ty] = ew[:, :, ty, 0].T
        enc_pair[CM:128, ty] = ew[:, :, ty, 1].T
        enc_single[:, ty] = ew[:, :, ty, 2].T
    enc_pair = enc_pair.astype(BF16)
    enc_single = enc_single.astype(BF16)
    enc_bias_a = bias_e.reshape(1, NK).astype(BF16)

    pack_base = np.zeros((128, 846), np.float32)
    pack_base[:, 0:128] = comp_lhsT.astype(np.float32).reshape(128, 128)
    pack_base[:, 128:428] = enc_pair.astype(np.float32).reshape(128, 300)
    pack_base[0:CM, 428:728] = enc_single.astype(np.float32).reshape(CM, 300)
    pack_base[0, 728:828] = enc_bias_a.astype(np.float32)[0]

    in_maps = []
    for core in range(NCORES):
        b, q = core // 4, core % 4
        # x_nat: rows [16q-1, 16q+17) cols padded, ch-partition layout
        xpad = np.zeros((C, 21, W + 4), np.float32)
        g0, g1 = 16 * q - 2, 16 * q + 19
        s0, s1 = max(g0, 0), min(g1, H)
        xpad[:, s0 - g0 : s1 - g0, 2 : W + 2] = x[b, :, s0:s1, :]
        x_nat = np.ascontiguousarray(
            xpad[:, 1 : 1 + NW1R, 2 : W + 2].reshape(2, 128, NW1R, W).transpose(
                1, 0, 2, 3
            )
        ).astype(BF16)

        # x_pix: partition (rho, j) rho-major; slab s = local rows 2s-2, 2s-1
        xs = np.zeros((128, 10, C), np.float32)
        for s in range(10):
            for rho in range(2):
                r = 16 * q - 2 + 2 * s + rho
                if 0 <= r < H:
                    xs[rho * 64 : rho * 64 + 64, s, :] = x[b, :, r, :].T
        x_pix = xs.astype(BF16)

        ridx = np.arange(NW1R)
        grows = 16 * q - 1 + ridx
        pack = pack_base.copy()
        pack[0:CM, 828:846] = ((grows >= 0) & (grows < H)).astype(np.float32)[None, :]

        m = {
            "x_nat": x_nat,
            "x_pix": x_pix,
            "pack": pack.astype(BF16),
            "comp_bias": comp_bias,
        }
        in_maps.append(m)
    return in_maps


def _assemble(results):
    out = np.zeros((B, C, H * SCALE, W * SCALE), np.float32)
    for core in range(NCORES):
        b, q = core // 4, core % 4
        arr = results[core]["out"].astype(np.float32)  # [8, 128, 4, C]
        # out pixel: row 32q + 4*lam + 2*rho + di, col 2*j + dj
        a = arr.reshape(8, 2, 64, 2, 2, C)  # [lam, rho, j, di, dj, c]
        blk = a.transpose(5, 0, 1, 3, 2, 4).reshape(C, 32, 128)
        out[b, :, 32 * q : 32 * q + 32, :] = blk
    return out


def kernel(**inputs):
    from concourse.bass_utils import run_bass_kernel_spmd

    if "nc" not in _compiled:
        _compiled["nc"] = _build_nc()
    nc = _compiled["nc"]
    in_maps = _host_inputs(**inputs)
    res = run_bass_kernel_spmd(nc, in_maps, core_ids=list(range(NCORES)))
    return _assemble(res.results)
